# revision 4
# baseline (speedup 1.0000x reference)
"""Trainium2 Bass kernel for nn_CrossAttention_9921374454346.

Cross-attention: y = softmax((x@Wq)(cond@Wk)^T / sqrt(dh)) (cond@Wv) @ Wo + b
Shapes: x [2,2048,1024], cond [2,2048,768], 16 heads x 64.

Sharding over 8 NeuronCores: data-parallel on batch (2) x tensor-parallel on
heads (4 groups of 4 heads).  Each core computes a partial output
y_partial[b] = softmax-attn(heads hg) @ Wo[rows of hg]; the host sums the 4
partials per batch (the "all-reduce" of row-parallel Wo) and adds the bias.

Per-core dataflow (everything fp16 on the PE, fp32 accumulation):
  - x/cond are cast f32->fp16 by SWDGE DMA into DRAM scratch, then
    DMA-transposed (xbar) into SBUF as x^T / cond^T (contraction dim on
    partitions).
  - Q^T = Wq^T x^T and K^T = Wk^T cond^T directly in transposed layout;
    V in natural [kv, d] layout, augmented with a ones column.
  - S^T tiles [kv,q] = (K^T tile)^T @ Q^T chunk (K=64 contraction; the two
    heads of a pair sit at partition bases 0/64 so their matmuls row-pack
    into the PE array concurrently).
  - P^T = exp(S^T / 8) on ACT (scale folded into the activation), fp16.
  - O'^T = [V|1]^T @ P^T accumulated over kv tiles -> rows 0:64 = unnormalized
    O^T, row 64 = softmax denominator (the ones-column trick).
  - normalize: recip(denom) -> gpsimd partition_broadcast -> DVE multiply.
  - y = O^T^T @ Wo via lhsT=O^T (natural), evict fp32, DMA out.
"""

import sys

import numpy as np

if "/opt/trn_rl_repo" not in sys.path:
    sys.path.insert(0, "/opt/trn_rl_repo")

SQ, SKV, DM, DC = 2048, 2048, 1024, 768
NH, DH = 16, 64
NCORES = 8
HPC = 4  # heads per core
DAL = HPC * DH  # 256: local attention dim per core
P = 128

_NC_CACHE = {}


def _build_nc():
    import concourse.tile as tile
    from concourse import bacc, mybir

    FP32 = mybir.dt.float32
    FP16 = mybir.dt.float16
    Exp = mybir.ActivationFunctionType.Exp

    KX, KC = DM // P, DC // P  # 8, 6 contraction tiles
    NKV = SKV // P  # 16 kv tiles
    QC = 4  # q chunks
    QW = SQ // QC  # 512
    MT = DAL // P  # 2 local-dattn tiles (= head pairs)
    EXPG = 3  # (j,par) units per exp group (3 PSUM banks)

    nc = bacc.Bacc(None, target_bir_lowering=False)
    x_d = nc.dram_tensor("xb", [SQ, DM], FP32, kind="ExternalInput")
    c_d = nc.dram_tensor("condb", [SKV, DC], FP32, kind="ExternalInput")
    wq_d = nc.dram_tensor("wq", [DM, DAL], FP32, kind="ExternalInput")
    wk_d = nc.dram_tensor("wk", [DC, DAL], FP32, kind="ExternalInput")
    wv_d = nc.dram_tensor("wv", [DC, DAL], FP32, kind="ExternalInput")
    wo_d = nc.dram_tensor("wo", [DAL, DM], FP32, kind="ExternalInput")
    y_d = nc.dram_tensor("y", [SQ, DM], FP32, kind="ExternalOutput")

    with tile.TileContext(nc) as tc:
        from contextlib import ExitStack

        with ExitStack() as ctx:
            big = ctx.enter_context(tc.tile_pool(name="big", bufs=1))
            pt_p = ctx.enter_context(tc.tile_pool(name="pt", bufs=3))
            ytmp = ctx.enter_context(tc.tile_pool(name="ytmp", bufs=3))
            ntmp = ctx.enter_context(tc.tile_pool(name="ntmp", bufs=2))
            drp = ctx.enter_context(tc.tile_pool(name="dram", bufs=1, space="DRAM"))
            psA = ctx.enter_context(tc.tile_pool(name="psA", bufs=2, space="PSUM"))
            psS = ctx.enter_context(tc.tile_pool(name="psS", bufs=2, space="PSUM"))
            psO = psA  # share the 2 banks: proj/Y psum and PV psum never overlap

            # persistent SBUF tensors
            xT = big.tile([P, KX, SQ], FP16)
            cT = big.tile([P, KC, SKV], FP16)
            QT = big.tile([P, MT, SQ], FP16)
            KT = big.tile([P, MT, SKV], FP16)
            OT = big.tile([P, MT, SQ], FP16)
            Vt = big.tile([P, NKV, HPC, DH + 1], FP16)
            wq_s = big.tile([P, KX, DAL], FP16)
            wk_s = big.tile([P, KC, DAL], FP16)
            wv_s = big.tile([P, KC, DAL], FP16)
            wo_s = big.tile([P, MT, DM], FP16)

            # weights: cast-DMA straight into SBUF (k-tile on partitions)
            nc.gpsimd.dma_start(out=wq_s, in_=wq_d.rearrange("(k p) n -> p k n", p=P))
            nc.gpsimd.dma_start(out=wk_s, in_=wk_d.rearrange("(k p) n -> p k n", p=P))
            nc.gpsimd.dma_start(out=wv_s, in_=wv_d.rearrange("(k p) n -> p k n", p=P))
            nc.gpsimd.dma_start(out=wo_s, in_=wo_d.rearrange("(k p) n -> p k n", p=P))
            # ones column for the denominator trick (col DH stays 1.0)
            nc.vector.memset(Vt, 1.0)

            # stage x/cond to fp16 DRAM (cast DMA), then xbar-transpose to SBUF
            x_h = drp.tile([SQ, DM], FP16)
            c_h = drp.tile([SKV, DC], FP16)
            for k2 in range(KX // 2):
                nc.gpsimd.dma_start(
                    out=x_h[:, k2 * 256 : (k2 + 1) * 256],
                    in_=x_d[:, k2 * 256 : (k2 + 1) * 256],
                )
            for k in range(KX):
                nc.sync.dma_start(
                    out=xT[:, k, :], in_=x_h[:, k * P : (k + 1) * P], transpose=True
                )
            for k2 in range(KC // 2):
                nc.gpsimd.dma_start(
                    out=c_h[:, k2 * 256 : (k2 + 1) * 256],
                    in_=c_d[:, k2 * 256 : (k2 + 1) * 256],
                )
            for k in range(KC):
                nc.sync.dma_start(
                    out=cT[:, k, :], in_=c_h[:, k * P : (k + 1) * P], transpose=True
                )

            # Q^T / K^T projections (transposed layout)
            for m in range(MT):
                for q in range(QC):
                    ps = psA.tile([P, QW], FP32, name="mm")
                    for k in range(KX):
                        nc.tensor.matmul(
                            ps,
                            wq_s[:, k, m * P : (m + 1) * P],
                            xT[:, k, q * QW : (q + 1) * QW],
                            start=(k == 0),
                            stop=(k == KX - 1),
                        )
                    nc.vector.tensor_copy(QT[:, m, q * QW : (q + 1) * QW], ps)
            for m in range(MT):
                for q in range(QC):
                    ps = psA.tile([P, QW], FP32, name="mm")
                    for k in range(KC):
                        nc.tensor.matmul(
                            ps,
                            wk_s[:, k, m * P : (m + 1) * P],
                            cT[:, k, q * QW : (q + 1) * QW],
                            start=(k == 0),
                            stop=(k == KC - 1),
                        )
                    nc.vector.tensor_copy(KT[:, m, q * QW : (q + 1) * QW], ps)

            # V in natural [kv, d] layout (+ ones column preset by memset)
            for j in range(NKV):
                ps = psA.tile([P, QW], FP32, name="mm")
                for k in range(KC):
                    nc.tensor.matmul(
                        ps[:, :DAL],
                        cT[:, k, j * P : (j + 1) * P],
                        wv_s[:, k, :],
                        start=(k == 0),
                        stop=(k == KC - 1),
                    )
                nc.vector.tensor_copy(
                    Vt[:, j, :, 0:DH],
                    ps[:, :DAL].rearrange("p (h d) -> p h d", h=HPC),
                )

            # attention
            scale = DH**-0.5
            units = [(j, par) for j in range(NKV) for par in range(2)]
            for m in range(MT):
                for q in range(QC):
                    po = [psO.tile([P, QW], FP32, name="mm") for _ in range(2)]
                    for g0 in range(0, len(units), EXPG):
                        gu = units[g0 : g0 + EXPG]
                        pss = psS.tile([P, EXPG, QW], FP32, name="ps_s")
                        for ui, (j, par) in enumerate(gu):
                            rb = par * 64
                            nc.tensor.matmul(
                                pss[:, ui, :],
                                KT[rb : rb + 64, m, j * P : (j + 1) * P],
                                QT[rb : rb + 64, m, q * QW : (q + 1) * QW],
                                start=True,
                                stop=True,
                            )
                        pt = pt_p.tile([P, EXPG, QW], FP16, name="pt")
                        nc.scalar.activation(
                            pt[:, : len(gu), :], pss[:, : len(gu), :], Exp, scale=scale
                        )
                        for ui, (j, par) in enumerate(gu):
                            hl = 2 * m + par
                            nc.tensor.matmul(
                                po[par][0 : DH + 1, :],
                                Vt[:, j, hl, :],
                                pt[:, ui, :],
                                start=(j == 0),
                                stop=(j == NKV - 1),
                            )
                    for par in range(2):
                        rb = par * 64
                        o_sb = ntmp.tile([DH + 1, QW], FP32, name="osb")
                        nc.vector.tensor_copy(o_sb, po[par][0 : DH + 1, :])
                        rc = ntmp.tile([1, QW], FP32, name="rc")
                        nc.vector.reciprocal(rc, o_sb[DH : DH + 1, :])
                        bc = ntmp.tile([64, QW], FP32, name="bc")
                        nc.gpsimd.partition_broadcast(bc, rc)
                        nc.vector.tensor_mul(
                            OT[rb : rb + 64, m, q * QW : (q + 1) * QW],
                            o_sb[0:DH, :],
                            bc,
                        )

            # output projection: y = O @ Wo (partial; host sums over head groups)
            for qt in range(SQ // P):
                for n2 in range(DM // QW):
                    ps = psA.tile([P, QW], FP32, name="mm")
                    for kt in range(MT):
                        nc.tensor.matmul(
                            ps,
                            OT[:, kt, qt * P : (qt + 1) * P],
                            wo_s[:, kt, n2 * QW : (n2 + 1) * QW],
                            start=(kt == 0),
                            stop=(kt == MT - 1),
                        )
                    ysb = ytmp.tile([P, QW], FP32, name="y")
                    nc.vector.tensor_copy(ysb, ps)
                    nc.sync.dma_start(
                        out=y_d[qt * P : (qt + 1) * P, n2 * QW : (n2 + 1) * QW],
                        in_=ysb,
                    )

    nc.compile()
    return nc


def get_nc():
    if "nc" not in _NC_CACHE:
        _NC_CACHE["nc"] = _build_nc()
    return _NC_CACHE["nc"]


def make_in_maps(x, cond, w_q, w_k, w_v, w_out):
    x = np.asarray(x, np.float32)
    cond = np.asarray(cond, np.float32)
    w_q = np.asarray(w_q, np.float32)
    w_k = np.asarray(w_k, np.float32)
    w_v = np.asarray(w_v, np.float32)
    w_out = np.asarray(w_out, np.float32)
    in_maps = []
    for c in range(NCORES):
        b, hg = divmod(c, NCORES // 2)
        sl = slice(hg * DAL, (hg + 1) * DAL)
        in_maps.append(
            {
                "xb": np.ascontiguousarray(x[b]),
                "condb": np.ascontiguousarray(cond[b]),
                "wq": np.ascontiguousarray(w_q[:, sl]),
                "wk": np.ascontiguousarray(w_k[:, sl]),
                "wv": np.ascontiguousarray(w_v[:, sl]),
                "wo": np.ascontiguousarray(w_out[sl, :]),
            }
        )
    return in_maps


def combine_outputs(results, b_out):
    out = np.zeros((2, SQ, DM), np.float32)
    for c in range(NCORES):
        out[c // (NCORES // 2)] += results[c]["y"]
    out += np.asarray(b_out, np.float32)[None, None, :]
    return out


def kernel(x, cond, w_q, w_k, w_v, w_out, b_out):
    from concourse.bass_utils import run_bass_kernel_spmd

    nc = get_nc()
    in_maps = make_in_maps(x, cond, w_q, w_k, w_v, w_out)
    res = run_bass_kernel_spmd(nc, in_maps, list(range(NCORES)))
    return combine_outputs(res.results, b_out)
